# revision 5
# baseline (speedup 1.0000x reference)
"""Multi-head attention (B=2, N=4096, C=512, H=8) on 8 TRN2 NeuronCores.

Sharding: core c handles batch b = c//4 and heads {2*(c%4), 2*(c%4)+1}
(data parallel over B, tensor parallel over heads).  Each core computes its
two heads' full attention plus their slice of the output projection; the
per-core [C, N] projection partials are summed per batch on the host
(the "proj all-reduce") and the projection bias is added there too.

Device-side math:
  qT = Wq_blk @ x^T + bq              bf16 [128(2 heads x 64), 4096]
  kT = Wk_blk @ x^T                   k bias dropped: softmax is invariant
                                      to per-query score offsets (exact)
  v  = Wv_blk @ x^T                   v bias folded into proj_b on host:
                                      out += Wp @ bv (exact)
  S^T(jt, ic) = K_jt @ Q_ic^T         bf16, scores transposed, heads packed
  P^T = exp(SCALE * S^T)              ScalarE -> fp8 p6 tiles
  O_raw^T, denom = [v|1|0pad] matmul  fp8 DoubleRow over j-tile pairs
  O^T = O_raw^T * (1/denom)           bf16 (denom = acc row 64)
  out^T partial = Wp_blk^T @ O^T      bf16, bf16 partials -> DRAM

attn@V uses fp8e4m3 + MatmulPerfMode.DoubleRow: two 128-deep j-tiles per
pass at half cost per output column.  The DR lhsT free dim must be a
multiple of 32, so vno is [64 v dims | ones | zero pad] = 96 wide; acc row
64 is the softmax denominator.  Only P and V are fp8 (quantizing x/w/
onorm/wp overshoots the 2e-2 error gate); everything else stays bf16.

ScalarE's exp stream is the hard floor (1 elem/lane/cycle @1.2GHz plus a
222-cycle per-instruction SBUF-access bubble).  The 512 (unit, head)
half-unit score blocks are exp'd in 171 instructions of 3 half-units
(1536 elems) each -- the largest PSUM allows: two 3-bank score slots
ping-pong, and the remaining 2 banks ("aux") host the QKV workspace
during the first ~58 units (all attn@V pairs are deferred to SBUF-resident
fp8 p6 tiles), then become the attn@V accumulators and the projection
workspace.  ~252us ScalarE busy; PE (~167us) paces K j-tiles, V quads and
Q chunks into each exp's slack through a deadline-driven micro-op queue,
with Q for i-chunk c produced during i-chunk c-1 and scores-feeding ops
always emitted ahead of the scores matmuls that read them.
"""

import os
from collections import deque

import numpy as np
import ml_dtypes

SC_BUFS = int(os.environ.get("SC_BUFS", "3"))
ACC_BUFS = int(os.environ.get("ACC_BUFS", "1"))
DEFER_S = int(os.environ.get("DEFER_S", "28"))  # attn-pair backlog target
DECAY = int(os.environ.get("DECAY", "6"))  # units per backlog-target step-down
DEFER_U = int(os.environ.get("DEFER_U", "58"))  # unit where backlog decay begins
DRAIN_END = int(os.environ.get("DRAIN_END", "12"))  # units before the end to finish draining
P6_BUFS = int(os.environ.get("P6_BUFS", "12"))
LOOKAHEAD = int(os.environ.get("LOOKAHEAD", "2"))
UNIT_BUDGET = float(os.environ.get("UNIT_BUDGET", "440"))

B, N, C = 2, 4096, 512
H, Dh = 8, 64
SCALE = Dh**-0.5
NCORES = 8
HPC = 2  # heads per core
ICW = 512  # i-chunk width
NIC = N // ICW  # 8
JTW = 128  # j-tile width
NJT = N // JTW  # 32
NPR = NJT // 2  # 16 j-tile pairs
VW = 96  # vno width: 64 v dims + ones col + zero pad (DR needs mult of 32)

_BF16 = ml_dtypes.bfloat16
_FP8 = ml_dtypes.float8_e4m3

_cached_nc = {}


def _build_nc(reps=1):
    import concourse.bacc as bacc
    import concourse.tile as tile
    import concourse.mybir as mybir

    f32 = mybir.dt.float32
    bf16 = mybir.dt.bfloat16
    fp8 = mybir.dt.float8e4
    Exp = mybir.ActivationFunctionType.Exp
    mult = mybir.AluOpType.mult
    DR = mybir.MatmulPerfMode.DoubleRow

    nc = bacc.Bacc("TRN2", target_bir_lowering=False, debug=False)

    xt_d = nc.dram_tensor("xt", [128, 4, N], bf16, kind="ExternalInput").ap()
    wqkv_d = nc.dram_tensor("wqkv", [128, 4, 3 * 128], bf16, kind="ExternalInput").ap()
    wp_d = nc.dram_tensor("wp", [128, C], bf16, kind="ExternalInput").ap()
    bq_d = nc.dram_tensor("bq", [128, 1], f32, kind="ExternalInput").ap()
    ident_d = nc.dram_tensor("ident", [128, 128], bf16, kind="ExternalInput").ap()
    out_d = nc.dram_tensor("out", [C, N], bf16, kind="ExternalOutput").ap()

    with tile.TileContext(nc) as tc:
        with (
            tc.tile_pool(name="ps", space="PSUM", bufs=2) as ps,
            tc.tile_pool(name="sp", bufs=2) as sp,
            tc.tile_pool(name="pe", bufs=1) as pe,
        ):
            # --- persistent SBUF tensors ---
            xt = pe.tile([128, 4, N], bf16, tag="xt", name="xt")

            def load_x():
                # chunk 0 rides the (idle) ACT-issued queue so it lands in
                # parallel with the weight DMAs on the sync queue.
                for i in range(NIC):
                    eng = nc.scalar if i == 0 else nc.sync
                    eng.dma_start(
                        out=xt[:, :, i * ICW : (i + 1) * ICW],
                        in_=xt_d[:, :, i * ICW : (i + 1) * ICW],
                    )
                    if i == 0:
                        # remaining weights queue behind x0
                        nc.scalar.dma_start(
                            out=wqkv[:, :, 256:384], in_=wqkv_d[:, :, 256:384]
                        )
                        nc.scalar.dma_start(out=ident[:], in_=ident_d[:, :])
                        nc.scalar.dma_start(out=wp[:], in_=wp_d[:, :])

            # q/k weight blocks land first so the prologue can start; the v
            # block, bias and projection weights follow the first x chunk.
            wqkv = pe.tile([128, 4, 3 * 128], bf16, tag="wqkv", name="wqkv")
            nc.sync.dma_start(out=wqkv[:, :, 0:256], in_=wqkv_d[:, :, 0:256])
            bq = pe.tile([128, 1], f32, tag="bq", name="bq")
            nc.sync.dma_start(out=bq[:], in_=bq_d[:, :])
            ident = pe.tile([128, 128], bf16, tag="ident", name="ident")
            wp = pe.tile([128, C], bf16, tag="wp", name="wp")

            ones1 = pe.tile([1, Dh], bf16, tag="ones1", name="ones1")
            nc.vector.memset(ones1[:], 1.0)
            qT = pe.tile([128, N], bf16, tag="qT", name="qT")
            kT = pe.tile([128, N], bf16, tag="kT", name="kT")
            vT = pe.tile([128, N], bf16, tag="vT", name="vT")
            # v + ones column + zero pad: [n-part, jt, head, 96] fp8
            vno = pe.tile([128, NJT, HPC, VW], fp8, tag="vno", name="vno")
            onorm = pe.tile([128, N], bf16, tag="onorm", name="onorm")

            nc.vector.memset(vno[:, :, :, Dh:VW], 0.0)
            nc.vector.memset(vno[:, :, :, Dh : Dh + 1], 1.0)

            # PE pre-warm: dummy matmuls while the input DMAs land, so the
            # first real matmuls run at full clock (PE ramps after ~3us).
            warm = pe.tile([128, 128], bf16, tag="warm", name="warm")
            nc.vector.memset(warm[:], 0.0)
            wps = ps.tile([128, ICW], f32, tag="aux", bufs=2, name="wps")
            for _ in range(14):
                nc.tensor.matmul(
                    wps[:, 0:128], lhsT=warm[:], rhs=warm[:], start=True, stop=True
                )

            # --- QKV production micro-ops ---
            def k_tile(jt):
                """kT j-tile (bias-free): 4 matmuls + DVE copy, ~470ns."""
                i, r = divmod(jt, 4)
                isl = slice(i * ICW + r * JTW, i * ICW + (r + 1) * JTW)
                t = ps.tile([128, ICW], f32, tag="aux", bufs=2, name="kps")
                for ct in range(4):
                    nc.tensor.matmul(
                        t[:, 0:JTW],
                        lhsT=wqkv[:, ct, 128:256],
                        rhs=xt[:, ct, isl],
                        start=(ct == 0),
                        stop=(ct == 3),
                    )
                nc.vector.tensor_copy(out=kT[:, isl], in_=t[:, 0:JTW])

            def v_tile(jt):
                """vno j-tile (bias-free): matmuls, copy, transpose, fp8 cast."""
                i, r = divmod(jt, 4)
                isl = slice(i * ICW + r * JTW, i * ICW + (r + 1) * JTW)
                t = ps.tile([128, ICW], f32, tag="aux", bufs=2, name="vps")
                for ct in range(4):
                    nc.tensor.matmul(
                        t[:, 0:JTW],
                        lhsT=wqkv[:, ct, 256:384],
                        rhs=xt[:, ct, isl],
                        start=(ct == 0),
                        stop=(ct == 3),
                    )
                nc.vector.tensor_copy(out=vT[:, isl], in_=t[:, 0:JTW])
                pst = ps.tile([128, 4, 128], bf16, tag="aux", bufs=2, name="pst")
                nc.tensor.transpose(pst[:, 0, :], vT[:, isl], ident[:])
                nc.vector.tensor_copy(
                    out=vno[:, jt, :, 0:Dh],
                    in_=pst[:, 0, :].rearrange("p (h d) -> p h d", h=HPC),
                )

            def v_quad(jt0):
                """Four vno j-tiles with one PSUM slot pair and two DVE copies
                (for the late tiles, where supply latency doesn't matter)."""
                i = jt0 // 4
                isl = slice(i * ICW, (i + 1) * ICW)
                t = ps.tile([128, ICW], f32, tag="aux", bufs=2, name="vps")
                for r in range(4):
                    for ct in range(4):
                        nc.tensor.matmul(
                            t[:, r * JTW : (r + 1) * JTW],
                            lhsT=wqkv[:, ct, 256:384],
                            rhs=xt[:, ct, i * ICW + r * JTW : i * ICW + (r + 1) * JTW],
                            start=(ct == 0),
                            stop=(ct == 3),
                        )
                nc.vector.tensor_copy(out=vT[:, isl], in_=t[:, 0:ICW])
                pst = ps.tile([128, 4, 128], bf16, tag="aux", bufs=2, name="pst")
                for r in range(4):
                    nc.tensor.transpose(
                        pst[:, r, :],
                        vT[:, i * ICW + r * JTW : i * ICW + (r + 1) * JTW],
                        ident[:],
                    )
                nc.vector.tensor_copy(
                    out=vno[:, jt0 : jt0 + 4, :, 0:Dh],
                    in_=pst[:].rearrange("p r (h d) -> p r h d", h=HPC),
                )

            qkv_state = {}

            def q_quarter(i, ct):
                """One contraction tile of the qT chunk (+bias on the last)."""
                isl = slice(i * ICW, (i + 1) * ICW)
                if ct == 0:
                    t = ps.tile([128, ICW], f32, tag="aux", bufs=2, name="qps")
                    qkv_state[i] = t
                else:
                    t = qkv_state[i]
                nc.tensor.matmul(
                    t[:, 0:ICW],
                    lhsT=wqkv[:, ct, 0:128],
                    rhs=xt[:, ct, isl],
                    start=(ct == 0),
                    stop=(ct == 3),
                )
                if ct == 3:
                    del qkv_state[i]
                    nc.vector.tensor_scalar_add(
                        out=qT[:, isl], in0=t[:, 0:ICW], scalar1=bq[:, 0:1]
                    )

            def emit_score_h(sc, off, hu):
                """One half-unit of scores into column block off of sc tile."""
                u, h = divmod(hu, 2)
                ic, jt = u // NJT, u % NJT
                hsl = slice(h * Dh, (h + 1) * Dh)
                nc.tensor.matmul(
                    sc[:, off * ICW : (off + 1) * ICW],
                    lhsT=kT[hsl, jt * JTW : (jt + 1) * JTW],
                    rhs=qT[hsl, ic * ICW : (ic + 1) * ICW],
                    start=True,
                    stop=True,
                )

            def emit_proj_cc(ic, cc, tail=False):
                isl = slice(ic * ICW, (ic + 1) * ICW)
                # in the tail the score slots are free: spread the four pp
                # tiles over both PSUM tags so the matmuls don't serialize on
                # the two aux banks
                if tail and cc % 2:
                    pp = ps.tile([128, 3 * ICW], f32, tag="sc", bufs=2, name="ppt")
                else:
                    pp = ps.tile([128, ICW], f32, tag="aux", bufs=2, name="pp")
                nc.tensor.matmul(
                    pp[:, 0:ICW],
                    lhsT=wp[:, cc * 128 : (cc + 1) * 128],
                    rhs=onorm[:, isl],
                    start=True,
                    stop=True,
                )
                st = sp.tile([128, ICW], bf16, tag="st", bufs=4, name="st")
                if tail and cc % 2:
                    nc.scalar.copy(out=st[:], in_=pp[:, 0:ICW])
                else:
                    nc.vector.tensor_copy(out=st[:], in_=pp[:, 0:ICW])
                # tail: split the output DMAs across both hwdge queues
                deng = nc.scalar if tail and cc % 2 else nc.sync
                deng.dma_start(out=out_d[cc * 128 : (cc + 1) * 128, isl], in_=st[:])

            # --- attention (software-pipelined, 1536-wide exps) ---
            # 512 half-units (unit u, head h) -> 171 exp instructions of 3
            # half-units each (the last covers 2).  Each exp is (1536+222)
            # cycles instead of 3/2 x (1024+222): ACT busy drops ~266->250us.
            # PSUM: two 3-bank score slots ping-pong; the 2 remaining banks
            # ("aux") host ALL qkv workspace during the first ~56 units (attn
            # pairs fully deferred), then become the attn@V accumulators and
            # the projection workspace.
            for _rep in range(reps):
                load_x()
                units = [(ic, jt) for ic in range(NIC) for jt in range(NJT)]
                NHU = 2 * len(units)
                NEXP = (NHU + 2) // 3
                # prologue: k tiles 0,1 + q chunk 0 unlock exp 0 (hu 0..2)
                k_tile(0)
                for ct in range(4):
                    q_quarter(0, ct)

                v_done = [0]

                def v_quad_counted(jt0):
                    v_quad(jt0)
                    v_done[0] = jt0 + 4

                # (cost_ns, deadline_unit, feeds_scores, fn)
                queue = deque()
                for jt in range(2, NJT):
                    queue.append((220, max(0, jt - 3), True, lambda jt=jt: k_tile(jt)))
                for i in range(1, NIC):
                    dl = 12 if i == 1 else 36 + 2 * i
                    for ct in range(4):
                        queue.append(
                            (215, dl + ct, True, lambda i=i, ct=ct: q_quarter(i, ct))
                        )
                for i, jt0 in enumerate(range(0, NJT, 4)):
                    queue.append(
                        (1070, 26 + 4 * i, False, lambda jt0=jt0: v_quad_counted(jt0))
                    )
                queue = deque(sorted(queue, key=lambda op: op[1]))

                accs = {}
                attn_q = deque()  # deferred attn@V pairs: (pair_index, p6_tile)
                proj_q = deque()  # deferred projection column blocks

                # scores slots: sc[k % 2] holds half-units 3k..3k+2
                sc_tiles = {}
                hu_emitted = [0]  # next half-unit to emit

                def ensure_scores(upto_hu):
                    while hu_emitted[0] < min(upto_hu, NHU):
                        hu = hu_emitted[0]
                        k, off = divmod(hu, 3)
                        if off == 0:
                            sc_tiles[k] = ps.tile(
                                [128, 3 * ICW], f32, tag="sc", bufs=2, name="sc"
                            )
                        emit_score_h(sc_tiles[k], off, hu)
                        hu_emitted[0] += 1

                def normalize(ic, tail=False):
                    isl = slice(ic * ICW, (ic + 1) * ICW)
                    abufs = []
                    for h in range(HPC):
                        ab = sp.tile(
                            [Dh + 1, ICW], f32, tag=f"ab{h}", bufs=2, name="ab"
                        )
                        acc_t = accs.pop((ic, h))
                        if tail and h == 1:
                            nc.scalar.copy(out=ab[:], in_=acc_t[0 : Dh + 1, :])
                        else:
                            nc.vector.tensor_copy(
                                out=ab[:], in_=acc_t[0 : Dh + 1, :]
                            )
                        abufs.append(ab)
                    for h in range(HPC):
                        ab = abufs[h]
                        rc = sp.tile([1, ICW], bf16, tag=f"rc{h}", bufs=2, name="rc")
                        with nc.allow_low_precision(
                            reason="1/denom feeds a bf16 broadcast"
                        ):
                            nc.vector.reciprocal(rc[:], ab[Dh : Dh + 1, :])
                        if tail:
                            rb = ps.tile(
                                [Dh, ICW], f32, tag="aux", bufs=2, name="rb"
                            )
                            nc.tensor.matmul(
                                rb[0:Dh, :],
                                lhsT=ones1[:],
                                rhs=rc[:],
                                start=True,
                                stop=True,
                            )
                            nc.vector.tensor_tensor(
                                out=onorm[h * Dh : (h + 1) * Dh, isl],
                                in0=ab[0:Dh, :],
                                in1=rb[0:Dh, :],
                                op=mult,
                            )
                        else:
                            rbs = sp.tile(
                                [Dh, ICW], bf16, tag=f"rb{h}", bufs=2, name="rbs"
                            )
                            nc.gpsimd.partition_broadcast(rbs[:], rc[:])
                            nc.vector.tensor_tensor(
                                out=onorm[h * Dh : (h + 1) * Dh, isl],
                                in0=ab[0:Dh, :],
                                in1=rbs[:],
                                op=mult,
                            )

                def norm_and_proj(ic):
                    if ic < NIC - 1:
                        normalize(ic)
                        for cc in range(4):
                            proj_q.append(lambda ic=ic, cc=cc: emit_proj_cc(ic, cc))
                    else:
                        normalize(ic, tail=True)
                        # tail: stage all four column blocks into one tile and
                        # ship a single DMA (HWDGE issue time is serialized)
                        isl = slice(ic * ICW, (ic + 1) * ICW)
                        st_all = sp.tile([128, 4, ICW], bf16, tag="sta", bufs=1, name="sta")
                        for cc in range(4):
                            if cc % 2:
                                pp = ps.tile([128, 3 * ICW], f32, tag="sc", bufs=2, name="ppt")
                            else:
                                pp = ps.tile([128, ICW], f32, tag="aux", bufs=2, name="pp")
                            nc.tensor.matmul(
                                pp[:, 0:ICW],
                                lhsT=wp[:, cc * 128 : (cc + 1) * 128],
                                rhs=onorm[:, isl],
                                start=True,
                                stop=True,
                            )
                            if cc % 2:
                                nc.scalar.copy(out=st_all[:, cc, :], in_=pp[:, 0:ICW])
                            else:
                                nc.vector.tensor_copy(out=st_all[:, cc, :], in_=pp[:, 0:ICW])
                        nc.sync.dma_start(
                            out=out_d.rearrange("(cc p) n -> p cc n", p=128)[:, :, isl],
                            in_=st_all[:],
                        )

                def pop_attn():
                    p, p6_ = attn_q.popleft()
                    a_ic, pr = divmod(p, NPR)
                    if pr == 0:
                        for h in range(HPC):
                            accs[(a_ic, h)] = ps.tile(
                                [128, ICW], f32, tag="aux", bufs=2, name=f"acc{h}"
                            )
                    s0 = (4 * p) % 12
                    for h in range(HPC):
                        nc.tensor.matmul(
                            accs[(a_ic, h)][0:VW, :],
                            lhsT=vno[:, 2 * pr : 2 * pr + 2, h, :],
                            rhs=p6_[:, s0 + h : s0 + h + 3 : 2, :],
                            start=(pr == 0),
                            stop=(pr == NPR - 1),
                            perf_mode=DR,
                        )
                    if pr == NPR - 1:
                        norm_and_proj(a_ic)

                p6 = None
                pushed = [0]  # next pair index to push
                ensure_scores(2)
                for k in range(NEXP):
                    hu0 = 3 * k
                    nh = min(3, NHU - hu0)
                    if k % 4 == 0:
                        p6 = sp.tile(
                            [128, 12, ICW], fp8, tag="p6", bufs=P6_BUFS, name="p6"
                        )
                    r0 = hu0 % 12
                    if k == 0:
                        # split the first exp 2+1 so it starts before k_tile(1)
                        sc0 = sc_tiles[0]
                        nc.scalar.activation(
                            p6[:, 0:2, :], sc0[:, 0 : 2 * ICW], Exp, scale=SCALE
                        )
                        k_tile(1)
                        ensure_scores(3)
                        nc.scalar.activation(
                            p6[:, 2:3, :], sc0[:, 2 * ICW : 3 * ICW], Exp, scale=SCALE
                        )
                        del sc_tiles[0]
                    else:
                        # ACT: one exp over 3 half-units -> fp8 p6 rows
                        nc.scalar.activation(
                            p6[:, r0 : r0 + nh, :],
                            sc_tiles.pop(k)[:, 0 : nh * ICW],
                            Exp,
                            scale=SCALE,
                        )
                    # push attn pairs wholly covered by exps so far (a pair
                    # never straddles p6 tiles, and its tile is always the
                    # one this exp wrote)
                    while 4 * pushed[0] + 3 <= hu0 + nh - 1:
                        attn_q.append((pushed[0], p6))
                        pushed[0] += 1
                    cur_unit = (hu0 + nh) // 2
                    # scores-feeding micro-ops (k/q) that are due go first
                    spent = 0.0
                    while queue and queue[0][1] <= cur_unit and queue[0][2]:
                        cost, _, _, fn = queue.popleft()
                        fn()
                        spent += cost
                    ensure_scores(hu0 + nh + 3 * LOOKAHEAD)
                    while queue and (
                        queue[0][1] <= cur_unit
                        or spent + queue[0][0] <= UNIT_BUDGET
                    ):
                        cost, _, _, fn = queue.popleft()
                        fn()
                        spent += cost
                    # attn@V pops: fully deferred while qkv owns the aux
                    # banks, then drained at up to max_pops per exp
                    s_now = max(
                        0,
                        min(
                            DEFER_S - max(0, cur_unit - DEFER_U) // DECAY,
                            (8 * NJT - DRAIN_END) - cur_unit,
                        ),
                    )
                    max_pops = 3 if cur_unit >= 7 * NJT else 2
                    pops = 0
                    while (
                        attn_q
                        and len(attn_q) > s_now
                        and pops < max_pops
                        and 2 * (attn_q[0][0] % NPR) + 2 <= v_done[0]
                        and not (attn_q[0][0] % NPR == 0 and (queue or proj_q))
                    ):
                        pop_attn()
                        pops += 1
                        spent += 213
                    if proj_q and spent < UNIT_BUDGET and not queue:
                        proj_q.popleft()()
                while attn_q:
                    pop_attn()
                while proj_q:
                    proj_q.popleft()()

    nc.compile()
    return nc


def get_nc(reps=1):
    if reps not in _cached_nc:
        _cached_nc[reps] = _build_nc(reps)
    return _cached_nc[reps]


def make_in_maps(x, qkv_w, qkv_b, proj_w):
    """Build the per-core input dicts (host-side sharding + layout prep)."""
    x = np.asarray(x, dtype=np.float32)
    qkv_w = np.asarray(qkv_w, dtype=np.float32)
    qkv_b = np.asarray(qkv_b, dtype=np.float32)
    proj_w = np.asarray(proj_w, dtype=np.float32)

    ident = np.eye(128, dtype=_BF16)
    in_maps = []
    for c in range(NCORES):
        b, j = divmod(c, 4)
        rq = slice(128 * j, 128 * (j + 1))
        rk = slice(512 + 128 * j, 512 + 128 * (j + 1))
        rv = slice(1024 + 128 * j, 1024 + 128 * (j + 1))
        xt = np.ascontiguousarray(
            x[b].T.reshape(4, 128, N).transpose(1, 0, 2)
        ).astype(_BF16)
        wq = qkv_w[rq].T.reshape(4, 128, 128).transpose(1, 0, 2)
        wk = qkv_w[rk].T.reshape(4, 128, 128).transpose(1, 0, 2)
        wv = qkv_w[rv].T.reshape(4, 128, 128).transpose(1, 0, 2)
        wqkv = np.ascontiguousarray(np.concatenate([wq, wk, wv], axis=2)).astype(_BF16)
        wp = np.ascontiguousarray(proj_w[:, rq].T).astype(_BF16)
        bqc = np.ascontiguousarray(qkv_b[rq][:, None]).astype(np.float32)
        in_maps.append(
            {"xt": xt, "wqkv": wqkv, "wp": wp, "bq": bqc, "ident": ident}
        )
    return in_maps


def gather_output(results, qkv_b, proj_w, proj_b):
    """Sum per-core partials per batch, transpose, add bias.

    The v bias is folded in here: out += proj_w @ bv (exact, since the
    attention weights sum to 1)."""
    qkv_b = np.asarray(qkv_b, dtype=np.float32)
    proj_w = np.asarray(proj_w, dtype=np.float32)
    proj_b = np.asarray(proj_b, dtype=np.float32)
    bias = proj_b + proj_w @ qkv_b[2 * C : 3 * C]
    out = np.empty((B, N, C), dtype=np.float32)
    for b in range(B):
        acc = np.zeros((C, N), dtype=np.float32)
        for j in range(4):
            acc += np.asarray(results[4 * b + j]["out"]).astype(np.float32)
        out[b] = acc.T + bias
    return out


def kernel(x, qkv_w, qkv_b, proj_w, proj_b):
    from concourse.bass_utils import run_bass_kernel_spmd

    nc = get_nc()
    in_maps = make_in_maps(x, qkv_w, qkv_b, proj_w)
    res = run_bass_kernel_spmd(nc, in_maps, list(range(NCORES)))
    return gather_output(res.results, qkv_b, proj_w, proj_b)


def run_traced(x, qkv_w, qkv_b, proj_w, proj_b, trace_cores=None):
    """Like kernel(), but profiles and returns (out, exec_time_ns, raw result)."""
    from concourse.bass_utils import run_bass_kernel_spmd

    nc = get_nc()
    in_maps = make_in_maps(x, qkv_w, qkv_b, proj_w)
    res = run_bass_kernel_spmd(
        nc, in_maps, list(range(NCORES)), trace=True, trace_cores=trace_cores
    )
    return gather_output(res.results, qkv_b, proj_w, proj_b), res.exec_time_ns, res


# revision 7
# speedup vs baseline: 1.0021x; 1.0021x over previous
"""Multi-head attention (B=2, N=4096, C=512, H=8) on 8 TRN2 NeuronCores.

Sharding: core c handles batch b = c//4 and heads {2*(c%4), 2*(c%4)+1}
(data parallel over B, tensor parallel over heads).  Each core computes its
two heads' full attention plus their slice of the output projection; the
per-core [C, N] projection partials are summed per batch on the host
(the "proj all-reduce") and the projection bias is added there too.

Device-side math:
  qT = Wq_blk @ x^T + bq              bf16 [128(2 heads x 64), 4096]
  kT = Wk_blk @ x^T                   k bias dropped: softmax is invariant
                                      to per-query score offsets (exact)
  v  = Wv_blk @ x^T                   v bias folded into proj_b on host:
                                      out += Wp @ bv (exact)
  S^T(jt, ic) = K_jt @ Q_ic^T         bf16, scores transposed, heads packed
  P^T = exp(SCALE * S^T)              ScalarE -> fp8 p6 tiles
  O_raw^T, denom = [v|1|0pad] matmul  fp8 DoubleRow over j-tile pairs
  O^T = O_raw^T * (1/denom)           bf16 (denom = acc row 64)
  out^T partial = Wp_blk^T @ O^T      bf16, bf16 partials -> DRAM

attn@V uses fp8e4m3 + MatmulPerfMode.DoubleRow: two 128-deep j-tiles per
pass at half cost per output column.  The DR lhsT free dim must be a
multiple of 32, so vno is [64 v dims | ones | zero pad] = 96 wide; acc row
64 is the softmax denominator.  Only P and V are fp8 (quantizing x/w/
onorm/wp overshoots the 2e-2 error gate); everything else stays bf16.

ScalarE's exp stream is the hard floor (1 elem/lane/cycle @1.2GHz plus a
222-cycle per-instruction SBUF-access bubble).  The 512 (unit, head)
half-unit score blocks are exp'd in 171 instructions of 3 half-units
(1536 elems) each -- the largest PSUM allows: two 3-bank score slots
ping-pong, and the remaining 2 banks ("aux") host the QKV workspace
during the first ~58 units (all attn@V pairs are deferred to SBUF-resident
fp8 p6 tiles), then become the attn@V accumulators and the projection
workspace.  ~252us ScalarE busy; PE (~167us) paces K j-tiles, V quads and
Q chunks into each exp's slack through a deadline-driven micro-op queue,
with Q for i-chunk c produced during i-chunk c-1 and scores-feeding ops
always emitted ahead of the scores matmuls that read them.
"""

import os
from collections import deque

import numpy as np
import ml_dtypes

SC_BUFS = int(os.environ.get("SC_BUFS", "3"))
ACC_BUFS = int(os.environ.get("ACC_BUFS", "1"))
DEFER_S = int(os.environ.get("DEFER_S", "28"))  # attn-pair backlog target
DECAY = int(os.environ.get("DECAY", "6"))  # units per backlog-target step-down
DEFER_U = int(os.environ.get("DEFER_U", "58"))  # unit where backlog decay begins
DRAIN_END = int(os.environ.get("DRAIN_END", "12"))  # units before the end to finish draining
P6_BUFS = int(os.environ.get("P6_BUFS", "12"))
LOOKAHEAD = int(os.environ.get("LOOKAHEAD", "2"))
UNIT_BUDGET = float(os.environ.get("UNIT_BUDGET", "440"))

B, N, C = 2, 4096, 512
H, Dh = 8, 64
SCALE = Dh**-0.5
NCORES = 8
HPC = 2  # heads per core
ICW = 512  # i-chunk width
NIC = N // ICW  # 8
JTW = 128  # j-tile width
NJT = N // JTW  # 32
NPR = NJT // 2  # 16 j-tile pairs
VW = 96  # vno width: 64 v dims + ones col + zero pad (DR needs mult of 32)

_BF16 = ml_dtypes.bfloat16
_FP8 = ml_dtypes.float8_e4m3

_cached_nc = {}


def _build_nc(reps=1):
    import concourse.bacc as bacc
    import concourse.tile as tile
    import concourse.mybir as mybir

    f32 = mybir.dt.float32
    bf16 = mybir.dt.bfloat16
    fp8 = mybir.dt.float8e4
    Exp = mybir.ActivationFunctionType.Exp
    mult = mybir.AluOpType.mult
    DR = mybir.MatmulPerfMode.DoubleRow

    nc = bacc.Bacc("TRN2", target_bir_lowering=False, debug=False)

    xt_d = nc.dram_tensor("xt", [128, 4, N], bf16, kind="ExternalInput").ap()
    wqkv_d = nc.dram_tensor("wqkv", [128, 4, 3 * 128], bf16, kind="ExternalInput").ap()
    wp_d = nc.dram_tensor("wp", [128, C], bf16, kind="ExternalInput").ap()
    bq_d = nc.dram_tensor("bq", [128, 1], f32, kind="ExternalInput").ap()
    ident_d = nc.dram_tensor("ident", [128, 128], bf16, kind="ExternalInput").ap()
    out_d = nc.dram_tensor("out", [C, N], bf16, kind="ExternalOutput").ap()

    with tile.TileContext(nc) as tc:
        with (
            tc.tile_pool(name="ps", space="PSUM", bufs=2) as ps,
            tc.tile_pool(name="sp", bufs=2) as sp,
            tc.tile_pool(name="pe", bufs=1) as pe,
        ):
            # --- persistent SBUF tensors ---
            xt = pe.tile([128, 4, N], bf16, tag="xt", name="xt")

            def load_x():
                # chunk 0 rides the (idle) ACT-issued queue so it lands in
                # parallel with the weight DMAs on the sync queue.
                for i in range(NIC):
                    eng = nc.scalar if i == 0 else nc.sync
                    eng.dma_start(
                        out=xt[:, :, i * ICW : (i + 1) * ICW],
                        in_=xt_d[:, :, i * ICW : (i + 1) * ICW],
                    )
                    if i == 0:
                        # remaining weights queue behind x0
                        nc.scalar.dma_start(
                            out=wqkv[:, :, 256:384], in_=wqkv_d[:, :, 256:384]
                        )
                        nc.scalar.dma_start(out=ident[:], in_=ident_d[:, :])
                        nc.scalar.dma_start(out=wp[:], in_=wp_d[:, :])

            # q/k weight blocks land first so the prologue can start; the v
            # block, bias and projection weights follow the first x chunk.
            wqkv = pe.tile([128, 4, 3 * 128], bf16, tag="wqkv", name="wqkv")
            nc.sync.dma_start(out=wqkv[:, :, 0:256], in_=wqkv_d[:, :, 0:256])
            bq = pe.tile([128, 1], f32, tag="bq", name="bq")
            nc.sync.dma_start(out=bq[:], in_=bq_d[:, :])
            ident = pe.tile([128, 128], bf16, tag="ident", name="ident")
            wp = pe.tile([128, C], bf16, tag="wp", name="wp")

            ones1 = pe.tile([1, Dh], bf16, tag="ones1", name="ones1")
            nc.vector.memset(ones1[:], 1.0)
            qT = pe.tile([128, N], bf16, tag="qT", name="qT")
            kT = pe.tile([128, N], bf16, tag="kT", name="kT")
            vT = pe.tile([128, N], bf16, tag="vT", name="vT")
            # v + ones column + zero pad: [n-part, jt, head, 96] fp8
            vno = pe.tile([128, NJT, HPC, VW], fp8, tag="vno", name="vno")
            onorm = pe.tile([128, N], bf16, tag="onorm", name="onorm")

            nc.vector.memset(vno[:, :, :, Dh:VW], 0.0)
            nc.vector.memset(vno[:, :, :, Dh : Dh + 1], 1.0)

            # PE pre-warm: dummy matmuls while the input DMAs land, so the
            # first real matmuls run at full clock (PE ramps after ~3us).
            warm = pe.tile([128, 128], bf16, tag="warm", name="warm")
            nc.vector.memset(warm[:], 0.0)
            wps = ps.tile([128, ICW], f32, tag="aux", bufs=2, name="wps")
            for _ in range(14):
                nc.tensor.matmul(
                    wps[:, 0:128], lhsT=warm[:], rhs=warm[:], start=True, stop=True
                )

            # --- QKV production micro-ops ---
            def k_tile(jt):
                """kT j-tile (bias-free): 4 matmuls + DVE copy, ~470ns."""
                i, r = divmod(jt, 4)
                isl = slice(i * ICW + r * JTW, i * ICW + (r + 1) * JTW)
                t = ps.tile([128, ICW], f32, tag="aux", bufs=2, name="kps")
                for ct in range(4):
                    nc.tensor.matmul(
                        t[:, 0:JTW],
                        lhsT=wqkv[:, ct, 128:256],
                        rhs=xt[:, ct, isl],
                        start=(ct == 0),
                        stop=(ct == 3),
                    )
                nc.vector.tensor_copy(out=kT[:, isl], in_=t[:, 0:JTW])

            def v_tile(jt):
                """vno j-tile (bias-free): matmuls, copy, transpose, fp8 cast."""
                i, r = divmod(jt, 4)
                isl = slice(i * ICW + r * JTW, i * ICW + (r + 1) * JTW)
                t = ps.tile([128, ICW], f32, tag="aux", bufs=2, name="vps")
                for ct in range(4):
                    nc.tensor.matmul(
                        t[:, 0:JTW],
                        lhsT=wqkv[:, ct, 256:384],
                        rhs=xt[:, ct, isl],
                        start=(ct == 0),
                        stop=(ct == 3),
                    )
                nc.vector.tensor_copy(out=vT[:, isl], in_=t[:, 0:JTW])
                pst = ps.tile([128, 4, 128], bf16, tag="aux", bufs=2, name="pst")
                nc.tensor.transpose(pst[:, 0, :], vT[:, isl], ident[:])
                nc.vector.tensor_copy(
                    out=vno[:, jt, :, 0:Dh],
                    in_=pst[:, 0, :].rearrange("p (h d) -> p h d", h=HPC),
                )

            def v_quad(jt0):
                """Four vno j-tiles with one PSUM slot pair and two DVE copies
                (for the late tiles, where supply latency doesn't matter)."""
                i = jt0 // 4
                isl = slice(i * ICW, (i + 1) * ICW)
                t = ps.tile([128, ICW], f32, tag="aux", bufs=2, name="vps")
                for r in range(4):
                    for ct in range(4):
                        nc.tensor.matmul(
                            t[:, r * JTW : (r + 1) * JTW],
                            lhsT=wqkv[:, ct, 256:384],
                            rhs=xt[:, ct, i * ICW + r * JTW : i * ICW + (r + 1) * JTW],
                            start=(ct == 0),
                            stop=(ct == 3),
                        )
                nc.vector.tensor_copy(out=vT[:, isl], in_=t[:, 0:ICW])
                pst = ps.tile([128, 4, 128], bf16, tag="aux", bufs=2, name="pst")
                for r in range(4):
                    nc.tensor.transpose(
                        pst[:, r, :],
                        vT[:, i * ICW + r * JTW : i * ICW + (r + 1) * JTW],
                        ident[:],
                    )
                nc.vector.tensor_copy(
                    out=vno[:, jt0 : jt0 + 4, :, 0:Dh],
                    in_=pst[:].rearrange("p r (h d) -> p r h d", h=HPC),
                )

            qkv_state = {}

            def q_quarter(i, ct):
                """One contraction tile of the qT chunk (+bias on the last)."""
                isl = slice(i * ICW, (i + 1) * ICW)
                if ct == 0:
                    t = ps.tile([128, ICW], f32, tag="aux", bufs=2, name="qps")
                    qkv_state[i] = t
                else:
                    t = qkv_state[i]
                nc.tensor.matmul(
                    t[:, 0:ICW],
                    lhsT=wqkv[:, ct, 0:128],
                    rhs=xt[:, ct, isl],
                    start=(ct == 0),
                    stop=(ct == 3),
                )
                if ct == 3:
                    del qkv_state[i]
                    nc.vector.tensor_scalar_add(
                        out=qT[:, isl], in0=t[:, 0:ICW], scalar1=bq[:, 0:1]
                    )

            def emit_score_h(sc, off, hu):
                """One half-unit of scores into column block off of sc tile."""
                u, h = divmod(hu, 2)
                ic, jt = u // NJT, u % NJT
                hsl = slice(h * Dh, (h + 1) * Dh)
                nc.tensor.matmul(
                    sc[:, off * ICW : (off + 1) * ICW],
                    lhsT=kT[hsl, jt * JTW : (jt + 1) * JTW],
                    rhs=qT[hsl, ic * ICW : (ic + 1) * ICW],
                    start=True,
                    stop=True,
                )

            def emit_proj_cc(ic, cc, tail=False):
                isl = slice(ic * ICW, (ic + 1) * ICW)
                # in the tail the score slots are free: spread the four pp
                # tiles over both PSUM tags so the matmuls don't serialize on
                # the two aux banks
                if tail and cc % 2:
                    pp = ps.tile([128, 3 * ICW], f32, tag="sc", bufs=2, name="ppt")
                else:
                    pp = ps.tile([128, ICW], f32, tag="aux", bufs=2, name="pp")
                nc.tensor.matmul(
                    pp[:, 0:ICW],
                    lhsT=wp[:, cc * 128 : (cc + 1) * 128],
                    rhs=onorm[:, isl],
                    start=True,
                    stop=True,
                )
                st = sp.tile([128, ICW], bf16, tag="st", bufs=4, name="st")
                if tail and cc % 2:
                    nc.scalar.copy(out=st[:], in_=pp[:, 0:ICW])
                else:
                    nc.vector.tensor_copy(out=st[:], in_=pp[:, 0:ICW])
                # tail: split the output DMAs across both hwdge queues
                deng = nc.scalar if tail and cc % 2 else nc.sync
                deng.dma_start(out=out_d[cc * 128 : (cc + 1) * 128, isl], in_=st[:])

            # --- attention (software-pipelined, 1536-wide exps) ---
            # 512 half-units (unit u, head h) -> 171 exp instructions of 3
            # half-units each (the last covers 2).  Each exp is (1536+222)
            # cycles instead of 3/2 x (1024+222): ACT busy drops ~266->250us.
            # PSUM: two 3-bank score slots ping-pong; the 2 remaining banks
            # ("aux") host ALL qkv workspace during the first ~56 units (attn
            # pairs fully deferred), then become the attn@V accumulators and
            # the projection workspace.
            for _rep in range(reps):
                load_x()
                units = [(ic, jt) for ic in range(NIC) for jt in range(NJT)]
                NHU = 2 * len(units)
                NEXP = (NHU + 2) // 3
                # prologue: k tiles 0,1 + q chunk 0 unlock exp 0 (hu 0..2)
                k_tile(0)
                for ct in range(4):
                    q_quarter(0, ct)

                v_done = [0]

                def v_quad_counted(jt0):
                    v_quad(jt0)
                    v_done[0] = jt0 + 4

                # (cost_ns, deadline_unit, feeds_scores, fn)
                queue = deque()
                for jt in range(2, NJT):
                    queue.append((220, max(0, jt - 3), True, lambda jt=jt: k_tile(jt)))
                for i in range(1, NIC):
                    dl = 12 if i == 1 else 36 + 2 * i
                    for ct in range(4):
                        queue.append(
                            (215, dl + ct, True, lambda i=i, ct=ct: q_quarter(i, ct))
                        )
                for i, jt0 in enumerate(range(0, NJT, 4)):
                    queue.append(
                        (1070, 26 + 4 * i, False, lambda jt0=jt0: v_quad_counted(jt0))
                    )
                queue = deque(sorted(queue, key=lambda op: op[1]))

                accs = {}
                attn_q = deque()  # deferred attn@V pairs: (pair_index, p6_tile)
                proj_q = deque()  # deferred projection column blocks

                # scores slots: sc[k % 2] holds half-units 3k..3k+2
                sc_tiles = {}
                hu_emitted = [0]  # next half-unit to emit

                def ensure_scores(upto_hu):
                    while hu_emitted[0] < min(upto_hu, NHU):
                        hu = hu_emitted[0]
                        k, off = divmod(hu, 3)
                        if off == 0:
                            sc_tiles[k] = ps.tile(
                                [128, 3 * ICW], f32, tag="sc", bufs=2, name="sc"
                            )
                        emit_score_h(sc_tiles[k], off, hu)
                        hu_emitted[0] += 1

                def normalize(ic, tail=False):
                    isl = slice(ic * ICW, (ic + 1) * ICW)
                    abufs = []
                    for h in range(HPC):
                        ab = sp.tile(
                            [Dh + 1, ICW], f32, tag=f"ab{h}", bufs=2, name="ab"
                        )
                        acc_t = accs.pop((ic, h))
                        if tail and h == 1:
                            nc.scalar.copy(out=ab[:], in_=acc_t[0 : Dh + 1, :])
                        else:
                            nc.vector.tensor_copy(
                                out=ab[:], in_=acc_t[0 : Dh + 1, :]
                            )
                        abufs.append(ab)
                    for h in range(HPC):
                        ab = abufs[h]
                        rc = sp.tile([1, ICW], bf16, tag=f"rc{h}", bufs=2, name="rc")
                        with nc.allow_low_precision(
                            reason="1/denom feeds a bf16 broadcast"
                        ):
                            nc.vector.reciprocal(rc[:], ab[Dh : Dh + 1, :])
                        if tail:
                            rb = ps.tile(
                                [Dh, ICW], f32, tag="aux", bufs=2, name="rb"
                            )
                            nc.tensor.matmul(
                                rb[0:Dh, :],
                                lhsT=ones1[:],
                                rhs=rc[:],
                                start=True,
                                stop=True,
                            )
                            nc.vector.tensor_tensor(
                                out=onorm[h * Dh : (h + 1) * Dh, isl],
                                in0=ab[0:Dh, :],
                                in1=rb[0:Dh, :],
                                op=mult,
                            )
                        else:
                            rbs = sp.tile(
                                [Dh, ICW], bf16, tag=f"rb{h}", bufs=2, name="rbs"
                            )
                            nc.gpsimd.partition_broadcast(rbs[:], rc[:])
                            nc.vector.tensor_tensor(
                                out=onorm[h * Dh : (h + 1) * Dh, isl],
                                in0=ab[0:Dh, :],
                                in1=rbs[:],
                                op=mult,
                            )

                def norm_and_proj(ic):
                    if ic < NIC - 1:
                        normalize(ic)
                        for cc in range(4):
                            proj_q.append(lambda ic=ic, cc=cc: emit_proj_cc(ic, cc))
                    else:
                        normalize(ic, tail=True)
                        # tail: stage all four column blocks into one tile and
                        # ship a single DMA (HWDGE issue time is serialized)
                        isl = slice(ic * ICW, (ic + 1) * ICW)
                        st_all = sp.tile([128, 4, ICW], bf16, tag="sta", bufs=1, name="sta")
                        for cc in range(4):
                            if cc % 2:
                                pp = ps.tile([128, 3 * ICW], f32, tag="sc", bufs=2, name="ppt")
                            else:
                                pp = ps.tile([128, ICW], f32, tag="aux", bufs=2, name="pp")
                            nc.tensor.matmul(
                                pp[:, 0:ICW],
                                lhsT=wp[:, cc * 128 : (cc + 1) * 128],
                                rhs=onorm[:, isl],
                                start=True,
                                stop=True,
                            )
                            if cc % 2:
                                nc.scalar.copy(out=st_all[:, cc, :], in_=pp[:, 0:ICW])
                            else:
                                nc.vector.tensor_copy(out=st_all[:, cc, :], in_=pp[:, 0:ICW])
                        # two sync-queue DMAs: the first ships while the
                        # cc2/cc3 copies finish
                        od = out_d.rearrange("(cc p) n -> p cc n", p=128)
                        nc.sync.dma_start(out=od[:, 0:2, isl], in_=st_all[:, 0:2, :])
                        nc.sync.dma_start(out=od[:, 2:4, isl], in_=st_all[:, 2:4, :])

                def pop_attn():
                    p, p6_ = attn_q.popleft()
                    a_ic, pr = divmod(p, NPR)
                    if pr == 0:
                        for h in range(HPC):
                            accs[(a_ic, h)] = ps.tile(
                                [128, ICW], f32, tag="aux", bufs=2, name=f"acc{h}"
                            )
                    s0 = (4 * p) % 12
                    for h in range(HPC):
                        nc.tensor.matmul(
                            accs[(a_ic, h)][0:VW, :],
                            lhsT=vno[:, 2 * pr : 2 * pr + 2, h, :],
                            rhs=p6_[:, s0 + h : s0 + h + 3 : 2, :],
                            start=(pr == 0),
                            stop=(pr == NPR - 1),
                            perf_mode=DR,
                        )
                    if pr == NPR - 1:
                        norm_and_proj(a_ic)

                p6 = None
                pushed = [0]  # next pair index to push
                ensure_scores(2)
                for k in range(NEXP):
                    hu0 = 3 * k
                    nh = min(3, NHU - hu0)
                    if k % 4 == 0:
                        p6 = sp.tile(
                            [128, 12, ICW], fp8, tag="p6", bufs=P6_BUFS, name="p6"
                        )
                    r0 = hu0 % 12
                    if k == 0:
                        # split the first exp 2+1 so it starts before k_tile(1)
                        sc0 = sc_tiles[0]
                        nc.scalar.activation(
                            p6[:, 0:2, :], sc0[:, 0 : 2 * ICW], Exp, scale=SCALE
                        )
                        k_tile(1)
                        ensure_scores(3)
                        nc.scalar.activation(
                            p6[:, 2:3, :], sc0[:, 2 * ICW : 3 * ICW], Exp, scale=SCALE
                        )
                        del sc_tiles[0]
                    else:
                        # ACT: one exp over 3 half-units -> fp8 p6 rows
                        nc.scalar.activation(
                            p6[:, r0 : r0 + nh, :],
                            sc_tiles.pop(k)[:, 0 : nh * ICW],
                            Exp,
                            scale=SCALE,
                        )
                    # push attn pairs wholly covered by exps so far (a pair
                    # never straddles p6 tiles, and its tile is always the
                    # one this exp wrote)
                    while 4 * pushed[0] + 3 <= hu0 + nh - 1:
                        attn_q.append((pushed[0], p6))
                        pushed[0] += 1
                    cur_unit = (hu0 + nh) // 2
                    # scores-feeding micro-ops (k/q) that are due go first
                    spent = 0.0
                    while queue and queue[0][1] <= cur_unit and queue[0][2]:
                        cost, _, _, fn = queue.popleft()
                        fn()
                        spent += cost
                    ensure_scores(hu0 + nh + 3 * LOOKAHEAD)
                    while queue and (
                        queue[0][1] <= cur_unit
                        or spent + queue[0][0] <= UNIT_BUDGET
                    ):
                        cost, _, _, fn = queue.popleft()
                        fn()
                        spent += cost
                    # attn@V pops: fully deferred while qkv owns the aux
                    # banks, then drained at up to max_pops per exp
                    s_now = max(
                        0,
                        min(
                            DEFER_S - max(0, cur_unit - DEFER_U) // DECAY,
                            (8 * NJT - DRAIN_END) - cur_unit,
                        ),
                    )
                    max_pops = 3 if cur_unit >= 7 * NJT else 2
                    pops = 0
                    while (
                        attn_q
                        and len(attn_q) > s_now
                        and pops < max_pops
                        and 2 * (attn_q[0][0] % NPR) + 2 <= v_done[0]
                        and not (attn_q[0][0] % NPR == 0 and (queue or proj_q))
                    ):
                        pop_attn()
                        pops += 1
                        spent += 213
                    if proj_q and spent < UNIT_BUDGET and not queue:
                        proj_q.popleft()()
                while attn_q:
                    pop_attn()
                while proj_q:
                    proj_q.popleft()()

    nc.compile()
    return nc


def get_nc(reps=1):
    if reps not in _cached_nc:
        _cached_nc[reps] = _build_nc(reps)
    return _cached_nc[reps]


def make_in_maps(x, qkv_w, qkv_b, proj_w):
    """Build the per-core input dicts (host-side sharding + layout prep)."""
    x = np.asarray(x, dtype=np.float32)
    qkv_w = np.asarray(qkv_w, dtype=np.float32)
    qkv_b = np.asarray(qkv_b, dtype=np.float32)
    proj_w = np.asarray(proj_w, dtype=np.float32)

    ident = np.eye(128, dtype=_BF16)
    in_maps = []
    for c in range(NCORES):
        b, j = divmod(c, 4)
        rq = slice(128 * j, 128 * (j + 1))
        rk = slice(512 + 128 * j, 512 + 128 * (j + 1))
        rv = slice(1024 + 128 * j, 1024 + 128 * (j + 1))
        xt = np.ascontiguousarray(
            x[b].T.reshape(4, 128, N).transpose(1, 0, 2)
        ).astype(_BF16)
        wq = qkv_w[rq].T.reshape(4, 128, 128).transpose(1, 0, 2)
        wk = qkv_w[rk].T.reshape(4, 128, 128).transpose(1, 0, 2)
        wv = qkv_w[rv].T.reshape(4, 128, 128).transpose(1, 0, 2)
        wqkv = np.ascontiguousarray(np.concatenate([wq, wk, wv], axis=2)).astype(_BF16)
        wp = np.ascontiguousarray(proj_w[:, rq].T).astype(_BF16)
        bqc = np.ascontiguousarray(qkv_b[rq][:, None]).astype(np.float32)
        in_maps.append(
            {"xt": xt, "wqkv": wqkv, "wp": wp, "bq": bqc, "ident": ident}
        )
    return in_maps


def gather_output(results, qkv_b, proj_w, proj_b):
    """Sum per-core partials per batch, transpose, add bias.

    The v bias is folded in here: out += proj_w @ bv (exact, since the
    attention weights sum to 1)."""
    qkv_b = np.asarray(qkv_b, dtype=np.float32)
    proj_w = np.asarray(proj_w, dtype=np.float32)
    proj_b = np.asarray(proj_b, dtype=np.float32)
    bias = proj_b + proj_w @ qkv_b[2 * C : 3 * C]
    out = np.empty((B, N, C), dtype=np.float32)
    for b in range(B):
        acc = np.zeros((C, N), dtype=np.float32)
        for j in range(4):
            acc += np.asarray(results[4 * b + j]["out"]).astype(np.float32)
        out[b] = acc.T + bias
    return out


def kernel(x, qkv_w, qkv_b, proj_w, proj_b):
    from concourse.bass_utils import run_bass_kernel_spmd

    nc = get_nc()
    in_maps = make_in_maps(x, qkv_w, qkv_b, proj_w)
    res = run_bass_kernel_spmd(nc, in_maps, list(range(NCORES)))
    return gather_output(res.results, qkv_b, proj_w, proj_b)


def run_traced(x, qkv_w, qkv_b, proj_w, proj_b, trace_cores=None):
    """Like kernel(), but profiles and returns (out, exec_time_ns, raw result)."""
    from concourse.bass_utils import run_bass_kernel_spmd

    nc = get_nc()
    in_maps = make_in_maps(x, qkv_w, qkv_b, proj_w)
    res = run_bass_kernel_spmd(
        nc, in_maps, list(range(NCORES)), trace=True, trace_cores=trace_cores
    )
    return gather_output(res.results, qkv_b, proj_w, proj_b), res.exec_time_ns, res
